# revision 1
# baseline (speedup 1.0000x reference)
"""Trainium2 Bass kernel for nn_NeighborhoodSearch (sparse_attention).

Sharding: 8 cores = (batch b in {0,1}) x (head-pair hp in {0..3}); each core
computes a full-[N, D] partial contribution of its 2 heads through its slice
of Wo; the host sums the 4 partials per batch (and transposes back).

v2 restructure vs baseline:
 - The 3x3 neighborhood sum (softmax weights are provably all-ones) runs on
   the PE as a block-banded matmul F = x2nat^T @ W, with W the [N, N] band
   matrix (incl. edge-padding corrections) built on the host and shipped as
   bf16 blocks.  This replaces ~130us of DVE/GpSimd shift-add work.
 - V is produced directly in natural [n, hd] layout (stationary = x1 block)
   so the 36 PE transposes + evac copies disappear.
 - No DRAM round trips: LN-stat and softmax-denominator broadcasts use
   ones-row PE matmuls into PSUM instead of DMA broadcast loops.
 - All inputs are host-packed partition-major so each is ONE dma_start.
"""

import sys

sys.path.insert(0, "/opt/trn_rl_repo")

import numpy as np

import concourse.bass as bass
import concourse.mybir as mybir
import concourse.tile as tile
from concourse.bass_utils import run_bass_kernel_spmd

# ---------------------------------------------------------------- constants
B = 2
N = 2304          # sequence length = 48*48
D = 768           # model dim
G = 48            # grid side
P = 128           # partitions
DC = D // P       # 6 feature chunks
HD = 96           # head dim
HPC = 2           # heads per core
NQB = 384         # n-block width (matmul moving free dim)
NB = N // NQB     # 6 n-blocks
NKC = N // P      # 18 key chunks
NO = N // P       # 18 box output blocks
EPS = 1e-5
QSCALE = HD ** -0.5

F32 = mybir.dt.float32
F32R = mybir.dt.float32r
BF16 = mybir.dt.bfloat16

ADD = mybir.AluOpType.add
SUB = mybir.AluOpType.subtract
MULT = mybir.AluOpType.mult
EXP = mybir.ActivationFunctionType.Exp
SQRT = mybir.ActivationFunctionType.Sqrt
IDENT = mybir.ActivationFunctionType.Identity


def _patch_tile_drain():
    """This container's walrus accepts at most 1 sync-wait per instruction
    (2 for EventSemaphore), but TileContext's final drain can carry several.
    Split the excess waits onto single-wait SP nops emitted after the drain
    (all complete before the all-engine barrier, so semantics are kept)."""
    if getattr(tile.TileContext, "_drain_patched", False):
        return
    from concourse.tile import ScopedClock

    def _drain_and_barrier(self, tick_clock, wait_clock):
        nc = self.nc
        drain_inst = nc.sync.drain()
        wait_clock.add_sem_waits(
            drain_inst.ins, ScopedClock({None: tick_clock.global_clock})
        )
        si = drain_inst.ins.sync_info
        waits = list(si.on_wait or [])
        if len(waits) > 1:
            si.on_wait = waits[:1]
            for w in waits[1:]:
                nop = nc.sync.nop(nofuse=True)
                nsi = nop.ins.sync_info
                if nsi is None:
                    nop.ins.sync_info = mybir.SyncInfo(on_wait=[w], on_update=[])
                else:
                    nsi.on_wait = (nsi.on_wait or []) + [w]
        nc.all_engine_barrier()
        popped = nc._tile_sem_poison_stack.pop()
        assert popped is self._sem_poison
        nc.clear_and_free_semaphores(list(self.sems.allocated().values()))
        nc.all_engine_barrier()

    tile.TileContext._drain_and_barrier = _drain_and_barrier
    tile.TileContext._drain_patched = True


def _split_multiwaits(nc):
    """This walrus supports at most 1 sync-wait per instruction; move excess
    waits onto single-wait NoOps inserted just before (same engine)."""
    for fn in nc.m.functions:
        for blk in fn.blocks:
            insts = list(blk.instructions)
            new = []
            changed = False
            for inst in insts:
                si = inst.sync_info
                if si is not None and si.on_wait and len(si.on_wait) > 1:
                    waits = list(si.on_wait)
                    for j, wcond in enumerate(waits[:-1]):
                        nop = mybir.InstNoOp(
                            name=f"{inst.name}-w{j}", engine=inst.engine,
                            ins=[], outs=[],
                            sync_info=mybir.SyncInfo(on_wait=[wcond],
                                                     on_update=[]))
                        new.append(nop)
                    si.on_wait = waits[-1:]
                    changed = True
                new.append(inst)
            if changed:
                blk.instructions = new


def build_nc(split_waits=True, reps=1):
    _patch_tile_drain()
    nc = bass.Bass("TRN2", target_bir_lowering=False, debug=False)

    x1t = nc.dram_tensor("x1t", [P, DC * N], F32R, kind="ExternalInput").ap()
    x2n = nc.dram_tensor("x2n", [P, NO * D], BF16, kind="ExternalInput").ap()
    wt = nc.dram_tensor("wt", [P, NO * 3 * P], BF16, kind="ExternalInput").ap()
    wkq = nc.dram_tensor("wkq", [P, DC * 2 * HPC * HD], F32R,
                         kind="ExternalInput").ap()
    wvp = nc.dram_tensor("wvp", [P, DC * 256], F32R, kind="ExternalInput").ap()
    wo = nc.dram_tensor("wo", [P, HPC * D], F32R, kind="ExternalInput").ap()
    cstr = nc.dram_tensor("cstr", [P, P + 2], F32R, kind="ExternalInput").ap()
    hc = nc.dram_tensor("hc", [P, 16], F32, kind="ExternalInput").ap()
    outp = nc.dram_tensor("outp", [D, N], F32, kind="ExternalOutput").ap()

    x1r = x1t.rearrange("p (c n) -> p c n", c=DC)
    x2r = x2n.rearrange("p (o d) -> p o d", o=NO)
    wtr = wt.rearrange("p (o j c) -> p o j c", o=NO, j=3)
    wkqr = wkq.rearrange("p (c q) -> p c q", c=DC)
    wvpr = wvp.rearrange("p (c q) -> p c q", c=DC)
    wor = wo.rearrange("p (h d) -> p h d", h=HPC)
    outr = outp.rearrange("(c p) n -> c p n", p=P)

    with tile.TileContext(nc) as tc:
      for _rep in range(reps):
        with tc.tile_pool(name="glob", bufs=1) as gp:
            # ---- persistent weights / consts (one DMA each)
            sb_wkq = gp.tile([P, DC, 2 * HPC * HD], F32R, tag="wkq")
            nc.scalar.dma_start(sb_wkq, wkqr)
            sb_wvp = gp.tile([P, DC, 256], F32R, tag="wvp")
            nc.scalar.dma_start(sb_wvp, wvpr)
            sb_wo = gp.tile([P, HPC, D], F32R, tag="wo")
            nc.scalar.dma_start(sb_wo, wor)
            sb_cr = gp.tile([P, P + 2], F32R, tag="cstr")
            nc.gpsimd.dma_start(sb_cr, cstr)
            sb_hc = gp.tile([P, 16], F32, tag="hc")
            nc.gpsimd.dma_start(sb_hc, hc)
            onesrow = sb_cr[0:1, 0:P]           # [1,128] ones (matmul bcast)
            ones = sb_cr[:, 0:1]
            zro = sb_cr[:, P:P + 1]
            eps_b = sb_hc[:, 0:1]

            # ---- persistent activations
            sb_q = gp.tile([P, HPC, N], F32R, tag="qT")
            sb_k = gp.tile([P, HPC, N], F32R, tag="kT")
            nc.gpsimd.tensor_copy(
                sb_q[HD:P, :, :],
                zro[HD:P, :, None].to_broadcast([P - HD, HPC, N]))
            nc.gpsimd.tensor_copy(
                sb_k[HD:P, :, :],
                zro[HD:P, :, None].to_broadcast([P - HD, HPC, N]))
            sb_v = gp.tile([P, NKC, HPC, HD + 1], F32R, tag="vnat")
            nc.gpsimd.tensor_copy(
                sb_v[:, :, :, HD:HD + 1],
                ones[:, :, None, None].to_broadcast([P, NKC, HPC, 1]))

            # ================================================== phase 1
            with tc.tile_pool(name="fpool", bufs=1) as fp:
                F = fp.tile([P, DC, N], F32R, tag="F")
                a_t = fp.tile([1, N], F32R, tag="a_t")   # rstd
                b_t = fp.tile([1, N], F32R, tag="b_t")   # mu*rstd

                # ---- box-sum on PE: F[:, c, o*128:...] = sum_j x2^T @ Wt
                with tc.tile_pool(name="x2p", bufs=1) as x2p, \
                     tc.tile_pool(name="ppf", bufs=3, space="PSUM") as ppf:
                    SPLITS = [(0, 2), (2, 8), (8, 18)]
                    wts, x2s = [], []
                    for si, (s0, s1) in enumerate(SPLITS):
                        w = x2p.tile([P, s1 - s0, 3, P], BF16,
                                     tag=f"wt{si}", name=f"wt{si}")
                        nc.sync.dma_start(w, wtr[:, s0:s1])
                        x = x2p.tile([P, s1 - s0, D], BF16,
                                     tag=f"x2{si}", name=f"x2{si}")
                        nc.sync.dma_start(x, x2r[:, s0:s1])
                        wts.append(w)
                        x2s.append(x)

                    def _seg(bp):
                        for si, (s0, s1) in enumerate(SPLITS):
                            if bp < s1:
                                return si, bp - s0
                        raise IndexError(bp)

                    def x2blk(bp, cs):
                        si, off = _seg(bp)
                        return x2s[si][:, off, cs]

                    def wtblk(o, j):
                        si, off = _seg(o)
                        return wts[si][:, off, j, :]

                    for o in range(NO):
                        psF = ppf.tile([P, DC, P], F32, tag="psF")
                        js = [j for j in range(3) if 0 <= o + j - 1 < NO]
                        for c in range(DC):
                            cs = slice(c * P, (c + 1) * P)
                            for ji, j in enumerate(js):
                                nc.tensor.matmul(
                                    psF[:, c, :],
                                    x2blk(o + j - 1, cs),
                                    wtblk(o, j),
                                    start=(ji == 0), stop=(ji == len(js) - 1))
                        if o % 2 == 0:
                            nc.scalar.activation(F[:, :, o * P:(o + 1) * P],
                                                 psF, IDENT)
                        else:
                            nc.vector.tensor_copy(F[:, :, o * P:(o + 1) * P],
                                                  psF)

                # ---- k/v proj + LN stats + q proj, software-pipelined:
                # qblk(nb-1) (bcast + q matmuls) is emitted after kv+stats(nb)
                # so the PE never waits on the serial DVE stats chain.
                with tc.tile_pool(name="x1p", bufs=2) as x1p, \
                     tc.tile_pool(name="sqp", bufs=2) as sqp, \
                     tc.tile_pool(name="abp", bufs=2) as abp, \
                     tc.tile_pool(name="ppj", bufs=2, space="PSUM") as ppj, \
                     tc.tile_pool(name="ppv", bufs=2, space="PSUM") as ppv, \
                     tc.tile_pool(name="pps", bufs=1, space="PSUM") as pps, \
                     tc.tile_pool(name="ppb", bufs=1, space="PSUM") as ppb:

                    def emit_kv(nb):
                        ns = slice(nb * NQB, (nb + 1) * NQB)
                        xb = x1p.tile([P, DC, NQB], F32R, tag="x1b")
                        deng = nc.scalar if nb < 2 else nc.sync
                        deng.dma_start(xb, x1r[:, :, ns])
                        for h in range(HPC):
                            psk = ppj.tile([HD, NQB], F32, tag="pkq")
                            for c in range(DC):
                                nc.tensor.matmul(psk,
                                                 sb_wkq[:, c, h * HD:(h + 1) * HD],
                                                 xb[:, c, :],
                                                 start=(c == 0), stop=(c == DC - 1))
                            nc.scalar.activation(sb_k[0:HD, h, ns], psk,
                                                 IDENT,
                                                 bias=sb_hc[0:HD, 1 + h:2 + h])
                        for t in range(NQB // P):
                            kc = nb * (NQB // P) + t
                            psv = ppv.tile([P, 256], F32, tag="pv")
                            for c in range(DC):
                                nc.tensor.matmul(psv,
                                                 xb[:, c, t * P:(t + 1) * P],
                                                 sb_wvp[:, c, :],
                                                 start=(c == 0), stop=(c == DC - 1))
                            nc.scalar.activation(
                                sb_v[:, kc, :, 0:HD],
                                psv[:, 0:HPC * HD].rearrange(
                                    "p (h d) -> p h d", h=HPC), IDENT)

                    def emit_sq(nb):
                        ns = slice(nb * NQB, (nb + 1) * NQB)
                        sq = sqp.tile([P, DC, NQB], F32R, tag="sq")
                        nc.gpsimd.tensor_tensor(sq, F[:, :, ns], F[:, :, ns],
                                                op=MULT)
                        return sq

                    def emit_stats(nb, sq):
                        ns = slice(nb * NQB, (nb + 1) * NQB)
                        psx = pps.tile([1, NQB], F32, tag="psx")
                        psq = pps.tile([1, NQB], F32, tag="psq")
                        for c in range(DC):
                            nc.tensor.matmul(psx, sb_cr[:, 0:1], F[:, c, ns],
                                             start=(c == 0), stop=(c == DC - 1))
                        for c in range(DC):
                            nc.tensor.matmul(psq, sb_cr[:, 0:1], sq[:, c, :],
                                             start=(c == 0), stop=(c == DC - 1))
                        av = a_t[0:1, ns]
                        bv = b_t[0:1, ns]
                        nc.vector.tensor_copy(bv, psx)            # bv = sx
                        nc.vector.scalar_tensor_tensor(
                            av, bv, 1.0 / (D * D), bv, op0=MULT, op1=MULT)
                        nc.vector.scalar_tensor_tensor(
                            av, psq, 1.0 / D, av, op0=MULT, op1=SUB)
                        nc.scalar.activation(av, av, SQRT,
                                             bias=sb_hc[0:1, 0:1])
                        with nc.allow_low_precision(reason="f32r==f32 bits"):
                            nc.vector.reciprocal(av, av)  # a = rstd
                        nc.vector.scalar_tensor_tensor(
                            bv, bv, 1.0 / D, av, op0=MULT, op1=MULT)  # b

                    def emit_qblk(nb):
                        ns = slice(nb * NQB, (nb + 1) * NQB)
                        psa = ppb.tile([P, NQB], F32, tag="psa")
                        psb = ppb.tile([P, NQB], F32, tag="psb")
                        nc.tensor.matmul(psa, onesrow, a_t[0:1, ns],
                                         start=True, stop=True)
                        nc.tensor.matmul(psb, onesrow, b_t[0:1, ns],
                                         start=True, stop=True)
                        ab = abp.tile([P, 2, NQB], F32R, tag="ab")
                        nc.vector.tensor_copy(ab[:, 0, :], psa)
                        nc.scalar.activation(ab[:, 1, :], psb, IDENT)
                        for h in range(HPC):
                            psq2 = ppj.tile([HD, NQB], F32, tag="pkq")
                            for c in range(DC):
                                nc.tensor.matmul(
                                    psq2,
                                    sb_wkq[:, c, (2 + h) * HD:(3 + h) * HD],
                                    F[:, c, ns],
                                    start=(c == 0), stop=(c == DC - 1))
                            qsl = sb_q[0:HD, h, ns]
                            nc.vector.tensor_tensor(qsl, psq2, ab[0:HD, 0, :],
                                                    op=MULT)
                            nc.vector.scalar_tensor_tensor(
                                qsl, ab[0:HD, 1, :], sb_hc[0:HD, 5 + h:6 + h],
                                qsl, op0=MULT, op1=ADD)
                            nc.vector.tensor_scalar(
                                qsl, qsl, sb_hc[0:HD, 7 + h:8 + h], None,
                                op0=ADD)

                    pend = None
                    sq_next = emit_sq(0)
                    for nb in range(NB):
                        emit_kv(nb)
                        sq_cur = sq_next
                        if nb + 1 < NB:
                            sq_next = emit_sq(nb + 1)
                        emit_stats(nb, sq_cur)
                        if pend is not None:
                            emit_qblk(pend)
                        pend = nb
                    emit_qblk(pend)

            # ================================================== attention
            with tc.tile_pool(name="att", bufs=2) as ap_, \
                 tc.tile_pool(name="ot", bufs=1) as otp, \
                 tc.tile_pool(name="den", bufs=2) as dnp, \
                 tc.tile_pool(name="ost", bufs=2) as osp, \
                 tc.tile_pool(name="ppk", bufs=2, space="PSUM") as ppk, \
                 tc.tile_pool(name="ppa", bufs=2, space="PSUM") as ppa, \
                 tc.tile_pool(name="ppd", bufs=1, space="PSUM") as ppd, \
                 tc.tile_pool(name="ppw", bufs=1, space="PSUM") as ppw:

                sb_o = otp.tile([P, HPC, N], F32R, tag="oT")
                nc.gpsimd.tensor_copy(
                    sb_o[HD:P, :, :],
                    zro[HD:P, :, None].to_broadcast([P - HD, HPC, N]))

                # 512-wide query blocks (4x512 + 1x256; one full PSUM bank
                # fp32). norm(i-1) trails one stage behind QK/AV(i) so the
                # in-order PE queue never waits on the DVE reciprocal.
                QBLKS = [(0, 512), (512, 512), (1024, 512), (1536, 512),
                         (2048, 256)]

                def emit_qkav(nb, h):
                    n0, w = QBLKS[nb]
                    ns = slice(n0, n0 + w)
                    att = ap_.tile([P, NKC, 512], F32R, tag="attT")
                    for kc2 in range(NKC // 2):
                        ps = ppk.tile([P, 2, 512], F32, tag="ps")
                        for j in range(2):
                            kc = kc2 * 2 + j
                            nc.tensor.matmul(
                                ps[:, j, 0:w],
                                sb_k[:, h, kc * P:(kc + 1) * P],
                                sb_q[:, h, ns], start=True, stop=True)
                        nc.scalar.activation(att[:, 2 * kc2:2 * kc2 + 2, 0:w],
                                             ps[:, :, 0:w], EXP)
                    po = ppa.tile([HD + 1, 512], F32, tag="po")
                    for kc in range(NKC):
                        nc.tensor.matmul(po[:, 0:w], sb_v[:, kc, h, :],
                                         att[:, kc, 0:w],
                                         start=(kc == 0), stop=(kc == NKC - 1))
                    d1 = dnp.tile([1, 512], F32R, tag="d1")
                    with nc.allow_low_precision(reason="f32r==f32 bits"):
                        nc.vector.reciprocal(d1[:, 0:w], po[HD:HD + 1, 0:w])
                    return po, d1

                def emit_norm(nb, h, po, d1):
                    n0, w = QBLKS[nb]
                    ns = slice(n0, n0 + w)
                    psd = ppd.tile([P, 512], F32, tag="psd")
                    nc.tensor.matmul(psd[:, 0:w], onesrow, d1[:, 0:w],
                                     start=True, stop=True)
                    pox = dnp.tile([HD, 512], F32, tag="pox")
                    nc.scalar.activation(pox[:, 0:w], po[0:HD, 0:w], IDENT)
                    nc.vector.tensor_tensor(sb_o[0:HD, h, ns], pox[:, 0:w],
                                            psd[0:HD, 0:w], op=MULT)

                def emit_outproj(nb):
                    n0, w = QBLKS[nb]
                    ns = slice(n0, n0 + w)
                    so = osp.tile([P, DC, 512], F32, tag="so")
                    for dc in range(DC):
                        pw = ppw.tile([P, 512], F32, tag="pw")
                        for h in range(HPC):
                            nc.tensor.matmul(pw[:, 0:w],
                                             sb_wo[:, h, dc * P:(dc + 1) * P],
                                             sb_o[:, h, ns],
                                             start=(h == 0), stop=(h == HPC - 1))
                        nc.vector.tensor_scalar(so[:, dc, 0:w], pw[:, 0:w],
                                                sb_hc[:, 9 + dc:10 + dc], None,
                                                op0=ADD)
                    nc.sync.dma_start(
                        outr[:, :, ns].rearrange("c p n -> p c n"),
                        so[:, :, 0:w])

                items = [(qb, h) for qb in range(len(QBLKS))
                         for h in range(HPC)]
                pend = None
                for i, (qb, h) in enumerate(items):
                    cur = (qb, h) + emit_qkav(qb, h)
                    if pend is not None:
                        emit_norm(*pend)
                        if pend[1] == HPC - 1:
                            emit_outproj(pend[0])
                    pend = cur
                emit_norm(*pend)
                emit_outproj(pend[0])
    if split_waits:
        _split_multiwaits(nc)
    return nc


def _build_wt():
    """Band matrix blocks: W[n', n] = multiplicity of neighbor n' for query n
    (padding slots replicate the first valid neighbor, reference order)."""
    rows = np.arange(N) // G
    cols = np.arange(N) % G
    offs = [(i, j) for i in (-1, 0, 1) for j in (-1, 0, 1)]
    W = np.zeros((N, N), np.float32)
    for n in range(N):
        r, c = rows[n], cols[n]
        first = -1
        npad = 0
        for dr, dc in offs:
            rr, cc = r + dr, c + dc
            if 0 <= rr < G and 0 <= cc < G:
                m = rr * G + cc
                W[m, n] += 1.0
                if first < 0:
                    first = m
            else:
                npad += 1
        if npad:
            W[first, n] += npad
    wt = np.zeros((NO, 3, P, P), np.float32)
    for o in range(NO):
        for j in range(3):
            bp = o + j - 1
            if 0 <= bp < NO:
                wt[o, j] = W[bp * P:(bp + 1) * P, o * P:(o + 1) * P]
    return wt.astype(np.float32)


_WT_CACHE = None


def make_core_inputs(inputs):
    """Host-side shard prep: slice/transpose weights, fold LN + q-scale."""
    global _WT_CACHE
    x1 = np.asarray(inputs["x1"], np.float32)
    x2 = np.asarray(inputs["x2"], np.float32)
    WqT = np.asarray(inputs["Wq"], np.float32).T
    WkT = np.asarray(inputs["Wk"], np.float32).T
    WvT = np.asarray(inputs["Wv"], np.float32).T
    WoT = np.asarray(inputs["Wo"], np.float32).T
    bq = np.asarray(inputs["bq"], np.float32)
    bk = np.asarray(inputs["bk"], np.float32)
    bv = np.asarray(inputs["bv"], np.float32)
    bo = np.asarray(inputs["bo"], np.float32)
    gamma = np.asarray(inputs["ln_gamma"], np.float32)
    beta = np.asarray(inputs["ln_beta"], np.float32)

    import ml_dtypes
    if _WT_CACHE is None:
        _WT_CACHE = _build_wt()
    wt_host = np.ascontiguousarray(
        _WT_CACHE.transpose(2, 0, 1, 3).reshape(P, -1)).astype(
            ml_dtypes.bfloat16)

    # partition-major packs
    x1t = []
    x2nat = []
    for b in range(B):
        xt = x1[b].T  # [D, N]
        x1t.append(np.ascontiguousarray(
            xt.reshape(DC, P, N).transpose(1, 0, 2).reshape(P, -1)))
        x2nat.append(np.ascontiguousarray(
            x2[b].reshape(NO, P, D).transpose(1, 0, 2).reshape(P, -1)
        ).astype(ml_dtypes.bfloat16))

    cstr_arr = np.ones((P, P + 2), np.float32)
    cstr_arr[:, P] = 0.0

    in_maps = []
    for core in range(8):
        b, hp = divmod(core, 4)
        sl = slice(HPC * HD * hp, HPC * HD * (hp + 1))
        wq_s = (WqT[:, sl] * QSCALE).astype(np.float32)
        wqg = (gamma[:, None] * wq_s).astype(np.float32)
        negg = (-wqg.sum(axis=0)).astype(np.float32)
        cq = (beta @ wq_s + bq[sl] * QSCALE).astype(np.float32)
        # wkq pack: [D, 192 wk | 192 wqg] -> [P, DC, 384]
        wkq_full = np.concatenate([WkT[:, sl], wqg], axis=1)  # [768, 384]
        wkq_host = np.ascontiguousarray(
            wkq_full.reshape(DC, P, 2 * HPC * HD).transpose(1, 0, 2)
            .reshape(P, -1))
        # wv padded to 256 moving cols
        wv_pad = np.zeros((D, 256), np.float32)
        wv_pad[:, 0:HPC * HD] = WvT[:, sl]
        wvp_host = np.ascontiguousarray(
            wv_pad.reshape(DC, P, 256).transpose(1, 0, 2).reshape(P, -1))
        # wo: [P(hd pad), h, D]
        wo_pad = np.zeros((P, HPC, D), np.float32)
        wo_pad[0:HD] = WoT[sl, :].reshape(HPC, HD, D).transpose(1, 0, 2)
        wo_host = np.ascontiguousarray(wo_pad.reshape(P, -1))
        # hc consts: 0 eps, 1-2 bk, 3 bv(flat later), 5-6 negg, 7-8 cq, 9-14 bo
        hc_arr = np.zeros((P, 16), np.float32)
        hc_arr[:, 0] = EPS
        hc_arr[0:HD, 1] = bk[sl][0:HD]
        hc_arr[0:HD, 2] = bk[sl][HD:2 * HD]
        # bv add: per-partition scalar applied to v-nat rows -> must be 0
        # unless bv is constant; v-nat layout has n on partitions, so a
        # per-partition scalar cannot represent per-hd bv. bv==0 in this
        # problem; assert and use 0 (col 3 stays zero).
        assert np.abs(bv).max() == 0.0, "v-nat path requires bv == 0"
        hc_arr[0:HD, 5] = negg[0:HD]
        hc_arr[0:HD, 6] = negg[HD:2 * HD]
        hc_arr[0:HD, 7] = cq[0:HD]
        hc_arr[0:HD, 8] = cq[HD:2 * HD]
        bo_eff = bo if hp == 0 else np.zeros_like(bo)
        hc_arr[:, 9:9 + DC] = bo_eff.reshape(DC, P).T
        in_maps.append({
            "x1t": x1t[b],
            "x2n": x2nat[b],
            "wt": wt_host,
            "wkq": wkq_host,
            "wvp": wvp_host,
            "wo": wo_host,
            "cstr": cstr_arr,
            "hc": hc_arr,
        })
    return in_maps


def kernel(**inputs):
    in_maps = make_core_inputs(inputs)
    nc = build_nc()
    res = run_bass_kernel_spmd(nc, in_maps, core_ids=list(range(8)))
    outs = [r["outp"] for r in res.results]
    out = np.empty((B, N, D), np.float32)
    for b in range(B):
        acc = outs[4 * b] + outs[4 * b + 1]
        acc += outs[4 * b + 2]
        acc += outs[4 * b + 3]
        out[b] = acc.T
    return out

